# revision 53
# baseline (speedup 1.0000x reference)
"""Trainium2 Bass kernel for the FallbackSSMKernel problem.

Computation (reference):
    xz = hidden @ W_in.T                     # (B,S,2*INNER)
    x, z = split(xz);  x -> (B,S,H,DH)
    h_n = A*h_{n-1} + 0.1*x_n  over chunks of 256 positions (per head)
    y = scan_out * sigmoid(z)
    out = y @ W_out.T                        # (B,S,DM)

Sharding: 8-way tensor-parallel by heads (14 heads / 896 inner dims per
core).  Each core computes the full token range for its head slice and a
partial out-projection (contraction over its 896 inner dims); bf16
partials are summed in fp32 on the host.

Device layout is fully transposed: activations live as (feature, token)
with features on SBUF partitions, so no transposes are ever needed on
device.  Per 512-token group the kernel runs:
  A) in_proj: PSUM tiles (x_s/z_s pairs) accumulated over 28 K-tiles of
     128.  The x branch (feeds the scan, error-sensitive) uses bf16;
     the z branch (only feeds a sigmoid gate) uses fp8e4m3 matmuls in
     DoubleRow perf mode (2 K-tiles per matmul, 2x PE throughput).  W_z
     is pre-scaled by 64 on the host to clear the fp8 subnormal range
     and the 1/64 is folded into the sigmoid's input scale.
  B) scan+gate directly from PSUM: h = h*A + x (DVE scalar_tensor_tensor,
     fp32 state), sig = sigmoid(z/64) (ACT), y = h*sig -> bf16 SBUF
  C) out_proj (transposed): outT[dm_tile, tokens] accumulated over the 7
     inner K-tiles, evicted via DVE to bf16 and DMA'd to DRAM.

All load DMAs ride the SP HWDGE queue and stores ride it too (stores on
the ACT queue measured ~200us slower).  The 0.1 scan input scale is
folded into the x-rows of W_in on the host.  Measured end-to-end
relative error 0.0169 (gate 2e-2), dominated by the fp8 z branch;
validated bit-exactly against a CPU emulation of the same datapath.
"""

import numpy as np
import ml_dtypes

B, S, DM = 2, 4096, 3584
H, DH = 112, 64
CHUNK = 256
INNER = H * DH
N_CORES = 8
HPC = H // N_CORES          # heads per core = 14
ISL = HPC * DH              # inner slice per core = 896
T = B * S                   # total tokens = 8192
G = 512                     # tokens per group

BF16 = ml_dtypes.bfloat16

_nc_cache = {}


def _patch_tile_drain():
    """Split the Tile end-of-kernel drain's semaphore waits across NOPs.

    The walrus build here rejects an InstDrain carrying more than a
    couple of sync waits ("Too many sync wait commands" in
    CoreV3GenImpl::setupSyncWait).  TileContext._drain_and_barrier
    attaches one wait per outstanding logical processor to the single
    drain, which trips that limit for any kernel that used a few DMA
    queues.  Emit one single-wait NOP per processor first so the drain
    itself needs no waits.
    """
    import concourse.tile as tile
    from concourse.vector_clock import ScopedClock, VectorClock

    if getattr(tile.TileContext, "_drain_split_patched", False):
        return

    def _drain_and_barrier(self, tick_clock, wait_clock):
        full = tick_clock.global_clock
        n = len(full)
        for proc in range(n):
            t = full[proc]
            if t > 0:
                vec = [0] * n
                vec[proc] = t
                nop = self.nc.sync.nop(nofuse=True, hint="drain_split")
                wait_clock.add_sem_waits(nop.ins, ScopedClock({None: VectorClock(vec)}))
        # No waits on the drain itself: SP executes the single-wait NOPs
        # above in order first, so every processor's final tick has been
        # observed before the drain runs.
        self.nc.sync.drain()
        self.nc.all_engine_barrier()
        popped = self.nc._tile_sem_poison_stack.pop()
        assert popped is self._sem_poison
        self.nc.clear_and_free_semaphores(list(self.sems.allocated().values()))
        self.nc.all_engine_barrier()

    tile.TileContext._drain_and_barrier = _drain_and_barrier
    tile.TileContext._drain_split_patched = True


def _split_excess_waits(nc, limit=1):
    """Hoist excess per-instruction semaphore waits onto inserted NOPs.

    The TRN2 64-byte instruction encoding carries at most `limit` sync
    waits; this walrus build hard-errors on more.  Tile can attach 3+
    waits to one instruction.  Hoisting the earliest waits onto
    preceding same-engine NOPs is semantics-preserving: semaphore
    values are monotonic, so waiting earlier on the same engine keeps
    the ordering guarantees.
    """
    import concourse.mybir as mybir

    counter = [0]
    for f in nc.m.functions:
        for blk in f.blocks:
            insts = blk.instructions
            new = []
            changed = False
            for inst in insts:
                si = inst.sync_info
                if si is not None and si.on_wait and len(si.on_wait) > limit:
                    waits = list(si.on_wait)
                    extra, keep = waits[:-limit], waits[-limit:]
                    for i in range(0, len(extra), limit):
                        chunk_w = extra[i:i + limit]
                        nop = mybir.InstNoOp(
                            name=f"WSPLIT-{counter[0]}", ins=[], outs=[]
                        )
                        counter[0] += 1
                        nop.engine = inst.engine
                        nop.sync_info = mybir.SyncInfo(
                            on_wait=chunk_w, on_update=[]
                        )
                        new.append(nop)
                    si.on_wait = keep
                    changed = True
                new.append(inst)
            if changed:
                blk.instructions = new
    return counter[0]


ZSCALE = 64.0     # z-branch fp8 weight pre-scale (folded out in the sigmoid)
XSCALE = 1024.0   # x-branch fp8 weight pre-scale (folded out through W_out)
# x-branch as fp8 hi/lo DoubleRow (3 matmuls per k-pair).  Measured ~480us
# SLOWER than bf16 on HW: DoubleRow disables fast-weight-load, and 42
# LDWEIGHTS of ~213ns per slab exceed the matmul savings.  Keep off.
USE_X_FP8 = False


def _build_bass(dm=DM, isl=ISL, tokens=T, n_batch=B, group=G, chunk=CHUNK,
                repeat=1, psa_bufs=4, psc_bufs=4, z_first=False,
                store_eng="sp", z_fp8=True, xz_interleave=False, oev_bufs=4,
                x_fp8=None, pipeline=True):
    """Build the per-core Bass module.

    Inputs (per core):
      hid_t  (dm, tokens)  bf16 : hidden_states, transposed
      w_in_t (dm, 2*isl)   bf16 : in_proj weight shard, transposed;
                                  cols [0,isl) are x-rows (pre-scaled by
                                  0.1), cols [isl,2*isl) are z-rows
      w_out_t(isl, dm)     bf16 : out_proj weight shard, transposed
      a_vec  (128, isl/128) f32 : per-inner-dim decay A = exp(-|A_log|)
    Output:
      out_t  (dm, tokens)  f32 : partial out-projection, transposed
    """
    import concourse.bass as bass
    import concourse.tile as tile
    import concourse.mybir as mybir

    _patch_tile_drain()

    ka = dm // 128            # in_proj contraction tiles
    kc = isl // 128           # out_proj contraction tiles / x slabs
    ng = tokens // group      # token groups
    gpb = ng // n_batch       # groups per batch
    kh = ka // 2              # k-tiles per hidden half-slab
    cpg = group // chunk      # chunks per group

    nc = bass.Bass("TRN2")
    dt = mybir.dt

    if x_fp8 is None:
        x_fp8 = USE_X_FP8
    assert not (x_fp8 and not z_fp8)
    hid_r = hidlo_r = wxhi_r = wxlo_r = None
    if not x_fp8:
        hid = nc.dram_tensor("hid_t", (dm, tokens), dt.bfloat16, kind="ExternalInput")
        hid_r = hid[:].rearrange("(k p) t -> p k t", p=128)
    if z_fp8:
        if x_fp8:
            # x-branch hi/lo fp8: hid8 doubles as the hi activation part
            wxhi = nc.dram_tensor("wxhi8_t", (dm, isl), dt.float8e4, kind="ExternalInput")
            wxlo = nc.dram_tensor("wxlo8_t", (dm, isl), dt.float8e4, kind="ExternalInput")
            hidlo = nc.dram_tensor("hidlo8_t", (dm, tokens), dt.float8e4, kind="ExternalInput")
            wxhi_r = wxhi[:].rearrange("(k p) m -> p k m", p=128)
            wxlo_r = wxlo[:].rearrange("(k p) m -> p k m", p=128)
            hidlo_r = hidlo[:].rearrange("(k p) t -> p k t", p=128)
        else:
            wx = nc.dram_tensor("wx_t", (dm, isl), dt.bfloat16, kind="ExternalInput")
            wx_r = wx[:].rearrange("(k p) m -> p k m", p=128)
        wz8 = nc.dram_tensor("wz8_t", (dm, isl), dt.float8e4, kind="ExternalInput")
        hid8 = nc.dram_tensor("hid8_t", (dm, tokens), dt.float8e4, kind="ExternalInput")
        wz8_r = wz8[:].rearrange("(k p) m -> p k m", p=128)
        hid8_r = hid8[:].rearrange("(k p) t -> p k t", p=128)
    else:
        w_in = nc.dram_tensor("w_in_t", (dm, 2 * isl), dt.bfloat16, kind="ExternalInput")
        win_r = w_in[:].rearrange("(k p) m -> p k m", p=128)
    w_out = nc.dram_tensor("w_out_t", (isl, dm), dt.bfloat16, kind="ExternalInput")
    a_vec = nc.dram_tensor("a_vec", (128, kc), dt.float32, kind="ExternalInput")
    # bf16 partials: halves store-DMA traffic; host sums 8 partials in fp32
    out = nc.dram_tensor("out_t", (dm, tokens), dt.bfloat16, kind="ExternalOutput")
    wout_r = w_out[:].rearrange("(k p) n -> p k n", p=128)

    with tile.TileContext(nc) as tc:
        with (
            tc.tile_pool(name="w_in", bufs=1) as p_win,
            tc.tile_pool(name="w_z8", bufs=1) as p_wz,
            tc.tile_pool(name="w_out", bufs=1) as p_wout,
            tc.tile_pool(name="consts", bufs=1) as p_const,
            tc.tile_pool(name="hid", bufs=2) as p_hid,
            tc.tile_pool(name="hid8", bufs=2) as p_hid8,
            tc.tile_pool(name="ysb", bufs=2) as p_y,
            tc.tile_pool(name="hstate", bufs=1) as p_h,
            tc.tile_pool(name="sig", bufs=4) as p_sig,
            tc.tile_pool(name="oev", bufs=oev_bufs) as p_oev,
            tc.tile_pool(name="psA", bufs=psa_bufs, space="PSUM") as p_psA,
            tc.tile_pool(name="psC", bufs=psc_bufs, space="PSUM") as p_psC,
        ):
            # Loads ride the SP HWDGE queue; wout + all phase-C stores ride
            # the ACT HWDGE queue so stores never head-of-line-block loads.
            st = nc.scalar if store_eng == "act" else nc.sync
            a_sb = p_const.tile([128, kc], dt.float32)
            nc.sync.dma_start(out=a_sb[:], in_=a_vec[:])
            if z_fp8 and x_fp8:
                wxhi_sb = p_win.tile([128, ka, isl], dt.float8e4)
                wxlo_sb = p_win.tile([128, ka, isl], dt.float8e4)
                for k in range(ka):
                    nc.sync.dma_start(out=wxhi_sb[:, k, :], in_=wxhi_r[:, k, :])
                    nc.sync.dma_start(out=wxlo_sb[:, k, :], in_=wxlo_r[:, k, :])
                wz_sb = p_wz.tile([128, ka, isl], dt.float8e4)
                for k in range(ka):
                    nc.sync.dma_start(out=wz_sb[:, k, :], in_=wz8_r[:, k, :])
                win_sb = ((wxhi_sb, wxlo_sb), wz_sb)
            elif z_fp8:
                wx_sb = p_win.tile([128, ka, isl], dt.bfloat16)
                for k in range(ka):
                    nc.sync.dma_start(out=wx_sb[:, k, :], in_=wx_r[:, k, :])
                wz_sb = p_wz.tile([128, ka, isl], dt.float8e4)
                for k in range(ka):
                    nc.sync.dma_start(out=wz_sb[:, k, :], in_=wz8_r[:, k, :])
                win_sb = (wx_sb, wz_sb)
            else:
                win_sb = p_win.tile([128, ka, 2 * isl], dt.bfloat16)
                for k in range(ka):
                    nc.sync.dma_start(out=win_sb[:, k, :], in_=win_r[:, k, :])
            wout_sb = p_wout.tile([128, kc, dm], dt.bfloat16)
            for k in range(kc):
                st.dma_start(out=wout_sb[:, k, :], in_=wout_r[:, k, :])
            h_sb = p_h.tile([128, kc, chunk], dt.float32)

            import contextlib
            rep_ctx = (
                tc.For_i(0, repeat, 1) if repeat > 1 else contextlib.nullcontext()
            )
            with rep_ctx:
                _emit_groups(
                    nc, tc, mybir, ng, gpb, ka, kh, kc, cpg, dm, isl, group, chunk,
                    hid_r, out, p_hid, p_y, p_sig, p_oev, p_psA, p_psC,
                    win_sb, wout_sb, a_sb, h_sb, z_first=z_first, store=st,
                    hid8_r=hid8_r if z_fp8 else None,
                    p_hid8=p_hid8 if z_fp8 else None,
                    xz_interleave=xz_interleave and z_fp8,
                    hidlo_r=hidlo_r, pipeline=pipeline,
                )

    n_split = _split_excess_waits(nc)
    if n_split:
        print(f"_split_excess_waits: inserted {n_split} NOPs", flush=True)
    return nc


def _emit_groups(nc, tc, mybir, ng, gpb, ka, kh, kc, cpg, dm, isl, group, chunk,
                 hid_r, out, p_hid, p_y, p_sig, p_oev, p_psA, p_psC,
                 win_sb, wout_sb, a_sb, h_sb, pipeline=False, z_first=False,
                 store=None, hid8_r=None, p_hid8=None, xz_interleave=False,
                 hidlo_r=None):
    z_fp8 = hid8_r is not None
    x_fp8 = hidlo_r is not None
    if z_fp8:
        wx_sb, wz_sb = win_sb
        if x_fp8:
            wxhi_sb, wxlo_sb = wx_sb
    # pipeline=True: out_proj of group g-1 is emitted after in_proj of
    # group g, so the PE never stalls on the DVE scan/gate chain at the
    # A->C boundary.  Measured -166us on HW (2026-08-09 session; the
    # earlier-session hang did not reproduce under the current schedule).
    dt = mybir.dt

    def emit_phase_c(g, y_sb):
        for m in range(dm // 128):
            po = p_psC.tile([128, group], dt.float32, tag="po")
            for k in range(kc):
                nc.tensor.matmul(
                    po,
                    wout_sb[:, k, m * 128:(m + 1) * 128],
                    y_sb[:, k, :],
                    start=(k == 0),
                    stop=(k == kc - 1),
                )
            oev = p_oev.tile([128, group], dt.bfloat16, tag="oev")
            nc.vector.tensor_copy(oev[:], po[:])
            (store or nc.sync).dma_start(
                out=out[m * 128:(m + 1) * 128, g * group:(g + 1) * group],
                in_=oev[:],
            )

    pending_c = None
    if True:
        if True:
            for g in range(ng):
                if g % gpb == 0:
                    # scan state resets at each batch boundary
                    nc.vector.memset(h_sb[:], 0.0)

                gs = slice(g * group, (g + 1) * group)
                hid_t = []
                for hh in range(2):
                    if x_fp8:
                        ht = p_hid.tile([128, kh, group], dt.float8e4, tag="hidlo")
                        nc.sync.dma_start(
                            out=ht[:], in_=hidlo_r[:, hh * kh:(hh + 1) * kh, gs],
                        )
                    else:
                        ht = p_hid.tile([128, kh, group], dt.bfloat16, tag="hid")
                        nc.sync.dma_start(
                            out=ht[:], in_=hid_r[:, hh * kh:(hh + 1) * kh, gs],
                        )
                    hid_t.append(ht)   # x_fp8: hid_t holds the LO activations
                hid8_t = []
                if z_fp8:
                    for hh in range(2):
                        ht8 = p_hid8.tile([128, kh, group], dt.float8e4, tag="hid8")
                        nc.sync.dma_start(
                            out=ht8[:], in_=hid8_r[:, hh * kh:(hh + 1) * kh, gs],
                        )
                        hid8_t.append(ht8)

                y_sb = p_y.tile([128, kc, group], dt.bfloat16)

                for s in range(kc):
                    px = p_psA.tile([128, group], dt.float32, tag="ps")
                    pz = p_psA.tile([128, group], dt.float32, tag="ps")

                    def emit_x():
                        if x_fp8:
                            # x = hi@Whi + hi@Wlo + lo@Whi (lo@Wlo dropped),
                            # all fp8 DoubleRow; hid_t holds the lo residuals.
                            terms = [(wxhi_sb, hid8_t), (wxlo_sb, hid8_t),
                                     (wxhi_sb, hid_t)]
                            n = len(terms)
                            for k2 in range(0, ka, 2):
                                hh, kk = divmod(k2, kh)
                                for t, (wsb, hsb) in enumerate(terms):
                                    nc.tensor.matmul(
                                        px,
                                        wsb[:, k2:k2 + 2, s * 128:(s + 1) * 128],
                                        hsb[hh][:, kk:kk + 2, :],
                                        start=(k2 == 0 and t == 0),
                                        stop=(k2 == ka - 2 and t == n - 1),
                                        perf_mode=mybir.MatmulPerfMode.DoubleRow,
                                    )
                            return
                        wsrc = wx_sb if z_fp8 else win_sb
                        for k in range(ka):
                            hh, kk = divmod(k, kh)
                            nc.tensor.matmul(
                                px,
                                wsrc[:, k, s * 128:(s + 1) * 128],
                                hid_t[hh][:, kk, :],
                                start=(k == 0),
                                stop=(k == ka - 1),
                            )

                    def emit_z():
                        if z_fp8:
                            # fp8 DoubleRow: one matmul covers 2 k-tiles
                            for k2 in range(0, ka, 2):
                                hh, kk = divmod(k2, kh)
                                nc.tensor.matmul(
                                    pz,
                                    wz_sb[:, k2:k2 + 2, s * 128:(s + 1) * 128],
                                    hid8_t[hh][:, kk:kk + 2, :],
                                    start=(k2 == 0),
                                    stop=(k2 == ka - 2),
                                    perf_mode=mybir.MatmulPerfMode.DoubleRow,
                                )
                        else:
                            for k in range(ka):
                                hh, kk = divmod(k, kh)
                                nc.tensor.matmul(
                                    pz,
                                    win_sb[:, k, isl + s * 128:isl + (s + 1) * 128],
                                    hid_t[hh][:, kk, :],
                                    start=(k == 0),
                                    stop=(k == ka - 1),
                                )

                    def emit_xz_interleaved():
                        # Pair two 213ns bf16 x-matmuls with one fp8 DoubleRow
                        # z-matmul (107ns compute, ~213ns LDWEIGHTS): the long
                        # z weight-load hides under the x matmuls and the PE
                        # never waits on the weight path.
                        for j in range(ka // 2):
                            for k in (2 * j, 2 * j + 1):
                                hh, kk = divmod(k, kh)
                                nc.tensor.matmul(
                                    px,
                                    wx_sb[:, k, s * 128:(s + 1) * 128],
                                    hid_t[hh][:, kk, :],
                                    start=(k == 0),
                                    stop=(k == ka - 1),
                                )
                            k2 = 2 * j
                            hh, kk = divmod(k2, kh)
                            nc.tensor.matmul(
                                pz,
                                wz_sb[:, k2:k2 + 2, s * 128:(s + 1) * 128],
                                hid8_t[hh][:, kk:kk + 2, :],
                                start=(k2 == 0),
                                stop=(k2 == ka - 2),
                                perf_mode=mybir.MatmulPerfMode.DoubleRow,
                            )

                    if xz_interleave:
                        emit_xz_interleaved()
                    elif z_first:
                        emit_z()
                        emit_x()
                    else:
                        emit_x()
                        emit_z()
                    sig = p_sig.tile([128, group], dt.bfloat16, tag="sig")
                    nc.scalar.activation(
                        sig[:], pz[:], mybir.ActivationFunctionType.Sigmoid,
                        scale=(1.0 / ZSCALE) if z_fp8 else 1.0,
                    )
                    for c in range(cpg):
                        cs = slice(c * chunk, (c + 1) * chunk)
                        nc.vector.scalar_tensor_tensor(
                            out=h_sb[:, s, :],
                            in0=h_sb[:, s, :],
                            scalar=a_sb[:, s:s + 1],
                            in1=px[:, cs],
                            op0=mybir.AluOpType.mult,
                            op1=mybir.AluOpType.add,
                        )
                        nc.vector.tensor_mul(y_sb[:, s, cs], h_sb[:, s, :], sig[:, cs])

                if pipeline:
                    # depth-1 software pipeline: group g-1's out_proj runs
                    # after group g's in_proj on the PE, so the PE never
                    # waits on the DVE scan/gate chain.
                    if pending_c is not None:
                        emit_phase_c(*pending_c)
                    pending_c = (g, y_sb)
                else:
                    emit_phase_c(g, y_sb)
            if pipeline and pending_c is not None:
                emit_phase_c(*pending_c)
                pending_c = None


def null_build(z_fp8=True, x_fp8=None):
    """Trivial kernel with the same I/O signature (per-call overhead cal)."""
    import concourse.bass as bass
    import concourse.tile as tile
    import concourse.mybir as mybir

    if x_fp8 is None:
        x_fp8 = USE_X_FP8
    _patch_tile_drain()
    dt = mybir.dt
    nc0 = bass.Bass("TRN2")
    if x_fp8:
        nc0.dram_tensor("wxhi8_t", (DM, ISL), dt.float8e4, kind="ExternalInput")
        nc0.dram_tensor("wxlo8_t", (DM, ISL), dt.float8e4, kind="ExternalInput")
        nc0.dram_tensor("hidlo8_t", (DM, T), dt.float8e4, kind="ExternalInput")
        nc0.dram_tensor("wz8_t", (DM, ISL), dt.float8e4, kind="ExternalInput")
        nc0.dram_tensor("hid8_t", (DM, T), dt.float8e4, kind="ExternalInput")
    elif z_fp8:
        nc0.dram_tensor("hid_t", (DM, T), dt.bfloat16, kind="ExternalInput")
        nc0.dram_tensor("wx_t", (DM, ISL), dt.bfloat16, kind="ExternalInput")
        nc0.dram_tensor("wz8_t", (DM, ISL), dt.float8e4, kind="ExternalInput")
        nc0.dram_tensor("hid8_t", (DM, T), dt.float8e4, kind="ExternalInput")
    else:
        nc0.dram_tensor("hid_t", (DM, T), dt.bfloat16, kind="ExternalInput")
        nc0.dram_tensor("w_in_t", (DM, 2 * ISL), dt.bfloat16, kind="ExternalInput")
    nc0.dram_tensor("w_out_t", (ISL, DM), dt.bfloat16, kind="ExternalInput")
    nc0.dram_tensor("a_vec", (128, ISL // 128), dt.float32, kind="ExternalInput")
    out0 = nc0.dram_tensor("out_t", (DM, T), dt.bfloat16, kind="ExternalOutput")
    with tile.TileContext(nc0) as tc:
        with tc.tile_pool(name="p", bufs=1) as p:
            z = p.tile([128, 16], dt.bfloat16)
            nc0.vector.memset(z[:], 0.0)
            nc0.sync.dma_start(out=out0[0:128, 0:16], in_=z[:])
    return nc0


# Results of the most recent device run (for test harness introspection).
last_result = None


def _prep_core_inputs(hidden_states, W_in, W_out, A_log, z_fp8=True,
                      x_fp8=None):
    """Host-side shard prep. Returns per-core in_maps."""
    if x_fp8 is None:
        x_fp8 = USE_X_FP8
    F8 = ml_dtypes.float8_e4m3
    hidT = np.ascontiguousarray(hidden_states.reshape(T, DM).T)
    hid_t = None if x_fp8 else hidT.astype(BF16)
    hid8_t = hidT.astype(F8) if z_fp8 else None
    if x_fp8:
        hidlo8_t = (
            hidT.astype(np.float32) - hid8_t.astype(np.float32)
        ).astype(F8)
    A_full = np.exp(-np.abs(A_log)).astype(np.float32)  # (H,)

    in_maps = []
    for c in range(N_CORES):
        isl_sl = slice(c * ISL, (c + 1) * ISL)
        # fold the 0.1 scan input scale into the x-rows of W_in
        w_x = W_in[:INNER][isl_sl] * np.float32(0.1)
        w_z = W_in[INNER:][isl_sl]
        # x_fp8: scan is linear in x, so scale W_x up into fp8 range and
        # fold the inverse scale into W_out (y carries the scale).
        wout_scale = np.float32(1.0 / XSCALE) if x_fp8 else np.float32(1.0)
        w_out_t = np.ascontiguousarray(
            (W_out[:, isl_sl] * wout_scale).T
        ).astype(BF16)  # (ISL, DM)
        a_col = np.repeat(A_full[c * HPC:(c + 1) * HPC], DH)  # (ISL,)
        a_vec = np.ascontiguousarray(
            a_col.reshape(ISL // 128, 128).T
        ).astype(np.float32)  # (128, ISL/128)
        if z_fp8 and x_fp8:
            ws = (w_x * np.float32(XSCALE)).astype(np.float32)
            whi = ws.astype(F8)
            wlo = (ws - whi.astype(np.float32)).astype(F8)
            m = {
                "hid8_t": hid8_t,
                "hidlo8_t": hidlo8_t,
                "wxhi8_t": np.ascontiguousarray(whi.T),
                "wxlo8_t": np.ascontiguousarray(wlo.T),
                "wz8_t": np.ascontiguousarray(
                    (w_z * np.float32(ZSCALE)).T
                ).astype(F8),
                "w_out_t": w_out_t,
                "a_vec": a_vec,
            }
        elif z_fp8:
            m = {
                "hid_t": hid_t,
                "hid8_t": hid8_t,
                "wx_t": np.ascontiguousarray(w_x.T).astype(BF16),
                "wz8_t": np.ascontiguousarray(
                    (w_z * np.float32(ZSCALE)).T
                ).astype(F8),
                "w_out_t": w_out_t,
                "a_vec": a_vec,
            }
        else:
            m = {
                "hid_t": hid_t,
                "w_in_t": np.ascontiguousarray(
                    np.concatenate([w_x, w_z], axis=0).T
                ).astype(BF16),
                "w_out_t": w_out_t,
                "a_vec": a_vec,
            }
        in_maps.append(m)
    return in_maps


def kernel(hidden_states, W_in, W_out, A_log):
    from concourse.bass_utils import run_bass_kernel_spmd

    global last_result

    if "nc" not in _nc_cache:
        _nc_cache["nc"] = _build_bass()
    nc = _nc_cache["nc"]

    hidden_states = np.asarray(hidden_states, dtype=np.float32)
    W_in = np.asarray(W_in, dtype=np.float32)
    W_out = np.asarray(W_out, dtype=np.float32)
    A_log = np.asarray(A_log, dtype=np.float32)

    in_maps = _prep_core_inputs(hidden_states, W_in, W_out, A_log)

    last_result = run_bass_kernel_spmd(nc, in_maps, core_ids=list(range(N_CORES)))

    acc = np.zeros((DM, T), dtype=np.float32)
    for r in last_result.results:
        acc += np.asarray(r["out_t"], dtype=np.float32)
    return np.ascontiguousarray(acc.T).reshape(B, S, DM)


if __name__ == "__main__":
    rng = np.random.default_rng(0)
    ins = {
        "hidden_states": rng.standard_normal((B, S, DM), dtype=np.float32),
        "W_in": (rng.standard_normal((2 * INNER, DM), dtype=np.float32) * 0.02),
        "W_out": (rng.standard_normal((DM, INNER), dtype=np.float32) * 0.02),
        "A_log": rng.standard_normal((H,), dtype=np.float32),
    }
    out = kernel(**ins)
    print(out.shape, out.dtype)



# revision 55
# speedup vs baseline: 1.1299x; 1.1299x over previous
"""Trainium2 Bass kernel for the FallbackSSMKernel problem.

Computation (reference):
    xz = hidden @ W_in.T                     # (B,S,2*INNER)
    x, z = split(xz);  x -> (B,S,H,DH)
    h_n = A*h_{n-1} + 0.1*x_n  over chunks of 256 positions (per head)
    y = scan_out * sigmoid(z)
    out = y @ W_out.T                        # (B,S,DM)

Sharding: 8-way tensor-parallel by heads (14 heads / 896 inner dims per
core).  Each core computes the full token range for its head slice and a
partial out-projection (contraction over its 896 inner dims); bf16
partials are summed in fp32 on the host.

Device layout is fully transposed: activations live as (feature, token)
with features on SBUF partitions, so no transposes are ever needed on
device.  Per 512-token group the kernel runs:
  A) in_proj: PSUM tiles (x_s/z_s pairs) accumulated over 28 K-tiles of
     128.  The x branch (feeds the scan, error-sensitive) uses bf16;
     the z branch (only feeds a sigmoid gate) uses fp8e4m3 matmuls in
     DoubleRow perf mode (2 K-tiles per matmul, 2x PE throughput).  W_z
     is pre-scaled by 64 on the host to clear the fp8 subnormal range
     and the 1/64 is folded into the sigmoid's input scale.
  B) scan+gate directly from PSUM: h = h*A + x (DVE scalar_tensor_tensor,
     fp32 state), sig = sigmoid(z/64) (ACT), y = h*sig -> bf16 SBUF
  C) out_proj (transposed): outT[dm_tile, tokens] accumulated over the 7
     inner K-tiles, evicted via DVE to bf16 and DMA'd to DRAM.

All load DMAs ride the SP HWDGE queue and stores ride it too (stores on
the ACT queue measured ~200us slower).  The 0.1 scan input scale is
folded into the x-rows of W_in on the host.  Measured end-to-end
relative error 0.0169 (gate 2e-2), dominated by the fp8 z branch;
validated bit-exactly against a CPU emulation of the same datapath.
"""

import numpy as np
import ml_dtypes

B, S, DM = 2, 4096, 3584
H, DH = 112, 64
CHUNK = 256
INNER = H * DH
N_CORES = 8
HPC = H // N_CORES          # heads per core = 14
ISL = HPC * DH              # inner slice per core = 896
T = B * S                   # total tokens = 8192
G = 512                     # tokens per group

BF16 = ml_dtypes.bfloat16

_nc_cache = {}


def _patch_tile_drain():
    """Split the Tile end-of-kernel drain's semaphore waits across NOPs.

    The walrus build here rejects an InstDrain carrying more than a
    couple of sync waits ("Too many sync wait commands" in
    CoreV3GenImpl::setupSyncWait).  TileContext._drain_and_barrier
    attaches one wait per outstanding logical processor to the single
    drain, which trips that limit for any kernel that used a few DMA
    queues.  Emit one single-wait NOP per processor first so the drain
    itself needs no waits.
    """
    import concourse.tile as tile
    from concourse.vector_clock import ScopedClock, VectorClock

    if getattr(tile.TileContext, "_drain_split_patched", False):
        return

    def _drain_and_barrier(self, tick_clock, wait_clock):
        full = tick_clock.global_clock
        n = len(full)
        for proc in range(n):
            t = full[proc]
            if t > 0:
                vec = [0] * n
                vec[proc] = t
                nop = self.nc.sync.nop(nofuse=True, hint="drain_split")
                wait_clock.add_sem_waits(nop.ins, ScopedClock({None: VectorClock(vec)}))
        # No waits on the drain itself: SP executes the single-wait NOPs
        # above in order first, so every processor's final tick has been
        # observed before the drain runs.
        self.nc.sync.drain()
        self.nc.all_engine_barrier()
        popped = self.nc._tile_sem_poison_stack.pop()
        assert popped is self._sem_poison
        self.nc.clear_and_free_semaphores(list(self.sems.allocated().values()))
        self.nc.all_engine_barrier()

    tile.TileContext._drain_and_barrier = _drain_and_barrier
    tile.TileContext._drain_split_patched = True


def _split_excess_waits(nc, limit=1):
    """Hoist excess per-instruction semaphore waits onto inserted NOPs.

    The TRN2 64-byte instruction encoding carries at most `limit` sync
    waits; this walrus build hard-errors on more.  Tile can attach 3+
    waits to one instruction.  Hoisting the earliest waits onto
    preceding same-engine NOPs is semantics-preserving: semaphore
    values are monotonic, so waiting earlier on the same engine keeps
    the ordering guarantees.
    """
    import concourse.mybir as mybir

    counter = [0]
    for f in nc.m.functions:
        for blk in f.blocks:
            insts = blk.instructions
            new = []
            changed = False
            for inst in insts:
                si = inst.sync_info
                if si is not None and si.on_wait and len(si.on_wait) > limit:
                    waits = list(si.on_wait)
                    extra, keep = waits[:-limit], waits[-limit:]
                    for i in range(0, len(extra), limit):
                        chunk_w = extra[i:i + limit]
                        nop = mybir.InstNoOp(
                            name=f"WSPLIT-{counter[0]}", ins=[], outs=[]
                        )
                        counter[0] += 1
                        nop.engine = inst.engine
                        nop.sync_info = mybir.SyncInfo(
                            on_wait=chunk_w, on_update=[]
                        )
                        new.append(nop)
                    si.on_wait = keep
                    changed = True
                new.append(inst)
            if changed:
                blk.instructions = new
    return counter[0]


ZSCALE = 64.0     # z-branch fp8 weight pre-scale (folded out in the sigmoid)
XSCALE = 1024.0   # x-branch fp8 weight pre-scale (folded out through W_out)
# x-branch as fp8 hi/lo DoubleRow (3 matmuls per k-pair).  Measured ~480us
# SLOWER than bf16 on HW: DoubleRow disables fast-weight-load, and 42
# LDWEIGHTS of ~213ns per slab exceed the matmul savings.  Keep off.
USE_X_FP8 = False


def _build_bass(dm=DM, isl=ISL, tokens=T, n_batch=B, group=G, chunk=CHUNK,
                repeat=1, psa_bufs=4, psc_bufs=4, z_first=False,
                store_eng="sp", z_fp8=True, xz_interleave=False, oev_bufs=4,
                x_fp8=None, pipeline=False):
    """Build the per-core Bass module.

    Inputs (per core):
      hid_t  (dm, tokens)  bf16 : hidden_states, transposed
      w_in_t (dm, 2*isl)   bf16 : in_proj weight shard, transposed;
                                  cols [0,isl) are x-rows (pre-scaled by
                                  0.1), cols [isl,2*isl) are z-rows
      w_out_t(isl, dm)     bf16 : out_proj weight shard, transposed
      a_vec  (128, isl/128) f32 : per-inner-dim decay A = exp(-|A_log|)
    Output:
      out_t  (dm, tokens)  f32 : partial out-projection, transposed
    """
    import concourse.bass as bass
    import concourse.tile as tile
    import concourse.mybir as mybir

    _patch_tile_drain()

    ka = dm // 128            # in_proj contraction tiles
    kc = isl // 128           # out_proj contraction tiles / x slabs
    ng = tokens // group      # token groups
    gpb = ng // n_batch       # groups per batch
    kh = ka // 2              # k-tiles per hidden half-slab
    cpg = group // chunk      # chunks per group

    nc = bass.Bass("TRN2")
    dt = mybir.dt

    if x_fp8 is None:
        x_fp8 = USE_X_FP8
    assert not (x_fp8 and not z_fp8)
    hid_r = hidlo_r = wxhi_r = wxlo_r = None
    if not x_fp8:
        hid = nc.dram_tensor("hid_t", (dm, tokens), dt.bfloat16, kind="ExternalInput")
        hid_r = hid[:].rearrange("(k p) t -> p k t", p=128)
    if z_fp8:
        if x_fp8:
            # x-branch hi/lo fp8: hid8 doubles as the hi activation part
            wxhi = nc.dram_tensor("wxhi8_t", (dm, isl), dt.float8e4, kind="ExternalInput")
            wxlo = nc.dram_tensor("wxlo8_t", (dm, isl), dt.float8e4, kind="ExternalInput")
            hidlo = nc.dram_tensor("hidlo8_t", (dm, tokens), dt.float8e4, kind="ExternalInput")
            wxhi_r = wxhi[:].rearrange("(k p) m -> p k m", p=128)
            wxlo_r = wxlo[:].rearrange("(k p) m -> p k m", p=128)
            hidlo_r = hidlo[:].rearrange("(k p) t -> p k t", p=128)
        else:
            wx = nc.dram_tensor("wx_t", (dm, isl), dt.bfloat16, kind="ExternalInput")
            wx_r = wx[:].rearrange("(k p) m -> p k m", p=128)
        wz8 = nc.dram_tensor("wz8_t", (dm, isl), dt.float8e4, kind="ExternalInput")
        hid8 = nc.dram_tensor("hid8_t", (dm, tokens), dt.float8e4, kind="ExternalInput")
        wz8_r = wz8[:].rearrange("(k p) m -> p k m", p=128)
        hid8_r = hid8[:].rearrange("(k p) t -> p k t", p=128)
    else:
        w_in = nc.dram_tensor("w_in_t", (dm, 2 * isl), dt.bfloat16, kind="ExternalInput")
        win_r = w_in[:].rearrange("(k p) m -> p k m", p=128)
    w_out = nc.dram_tensor("w_out_t", (isl, dm), dt.bfloat16, kind="ExternalInput")
    a_vec = nc.dram_tensor("a_vec", (128, kc), dt.float32, kind="ExternalInput")
    # bf16 partials: halves store-DMA traffic; host sums 8 partials in fp32
    out = nc.dram_tensor("out_t", (dm, tokens), dt.bfloat16, kind="ExternalOutput")
    wout_r = w_out[:].rearrange("(k p) n -> p k n", p=128)

    with tile.TileContext(nc) as tc:
        with (
            tc.tile_pool(name="w_in", bufs=1) as p_win,
            tc.tile_pool(name="w_z8", bufs=1) as p_wz,
            tc.tile_pool(name="w_out", bufs=1) as p_wout,
            tc.tile_pool(name="consts", bufs=1) as p_const,
            tc.tile_pool(name="hid", bufs=2) as p_hid,
            tc.tile_pool(name="hid8", bufs=2) as p_hid8,
            tc.tile_pool(name="ysb", bufs=2) as p_y,
            tc.tile_pool(name="hstate", bufs=1) as p_h,
            tc.tile_pool(name="sig", bufs=4) as p_sig,
            tc.tile_pool(name="oev", bufs=oev_bufs) as p_oev,
            tc.tile_pool(name="psA", bufs=psa_bufs, space="PSUM") as p_psA,
            tc.tile_pool(name="psC", bufs=psc_bufs, space="PSUM") as p_psC,
        ):
            # Loads ride the SP HWDGE queue; wout + all phase-C stores ride
            # the ACT HWDGE queue so stores never head-of-line-block loads.
            st = nc.scalar if store_eng == "act" else nc.sync
            a_sb = p_const.tile([128, kc], dt.float32)
            nc.sync.dma_start(out=a_sb[:], in_=a_vec[:])
            if z_fp8 and x_fp8:
                wxhi_sb = p_win.tile([128, ka, isl], dt.float8e4)
                wxlo_sb = p_win.tile([128, ka, isl], dt.float8e4)
                for k in range(ka):
                    nc.sync.dma_start(out=wxhi_sb[:, k, :], in_=wxhi_r[:, k, :])
                    nc.sync.dma_start(out=wxlo_sb[:, k, :], in_=wxlo_r[:, k, :])
                wz_sb = p_wz.tile([128, ka, isl], dt.float8e4)
                for k in range(ka):
                    nc.sync.dma_start(out=wz_sb[:, k, :], in_=wz8_r[:, k, :])
                win_sb = ((wxhi_sb, wxlo_sb), wz_sb)
            elif z_fp8:
                wx_sb = p_win.tile([128, ka, isl], dt.bfloat16)
                for k in range(ka):
                    nc.sync.dma_start(out=wx_sb[:, k, :], in_=wx_r[:, k, :])
                wz_sb = p_wz.tile([128, ka, isl], dt.float8e4)
                for k in range(ka):
                    nc.sync.dma_start(out=wz_sb[:, k, :], in_=wz8_r[:, k, :])
                win_sb = (wx_sb, wz_sb)
            else:
                win_sb = p_win.tile([128, ka, 2 * isl], dt.bfloat16)
                for k in range(ka):
                    nc.sync.dma_start(out=win_sb[:, k, :], in_=win_r[:, k, :])
            wout_sb = p_wout.tile([128, kc, dm], dt.bfloat16)
            for k in range(kc):
                st.dma_start(out=wout_sb[:, k, :], in_=wout_r[:, k, :])
            h_sb = p_h.tile([128, kc, chunk], dt.float32)

            import contextlib
            rep_ctx = (
                tc.For_i(0, repeat, 1) if repeat > 1 else contextlib.nullcontext()
            )
            with rep_ctx:
                _emit_groups(
                    nc, tc, mybir, ng, gpb, ka, kh, kc, cpg, dm, isl, group, chunk,
                    hid_r, out, p_hid, p_y, p_sig, p_oev, p_psA, p_psC,
                    win_sb, wout_sb, a_sb, h_sb, z_first=z_first, store=st,
                    hid8_r=hid8_r if z_fp8 else None,
                    p_hid8=p_hid8 if z_fp8 else None,
                    xz_interleave=xz_interleave and z_fp8,
                    hidlo_r=hidlo_r, pipeline=pipeline,
                )

    n_split = _split_excess_waits(nc)
    if n_split:
        print(f"_split_excess_waits: inserted {n_split} NOPs", flush=True)
    return nc


def _emit_groups(nc, tc, mybir, ng, gpb, ka, kh, kc, cpg, dm, isl, group, chunk,
                 hid_r, out, p_hid, p_y, p_sig, p_oev, p_psA, p_psC,
                 win_sb, wout_sb, a_sb, h_sb, pipeline=False, z_first=False,
                 store=None, hid8_r=None, p_hid8=None, xz_interleave=False,
                 hidlo_r=None):
    z_fp8 = hid8_r is not None
    x_fp8 = hidlo_r is not None
    if z_fp8:
        wx_sb, wz_sb = win_sb
        if x_fp8:
            wxhi_sb, wxlo_sb = wx_sb
    # pipeline=True: out_proj of group g-1 is emitted after in_proj of
    # group g, so the PE never stalls on the DVE scan/gate chain at the
    # A->C boundary.  No hang under the current schedule, but long-run
    # timing is bimodal on HW (1.88ms..2.38ms/iter across sessions,
    # likely a sustained-power/HAM interaction) while pipeline=False is
    # stable at ~2.02-2.06ms.  Keep off: stable beats occasionally-fast.
    dt = mybir.dt

    def emit_phase_c(g, y_sb):
        for m in range(dm // 128):
            po = p_psC.tile([128, group], dt.float32, tag="po")
            for k in range(kc):
                nc.tensor.matmul(
                    po,
                    wout_sb[:, k, m * 128:(m + 1) * 128],
                    y_sb[:, k, :],
                    start=(k == 0),
                    stop=(k == kc - 1),
                )
            oev = p_oev.tile([128, group], dt.bfloat16, tag="oev")
            nc.vector.tensor_copy(oev[:], po[:])
            (store or nc.sync).dma_start(
                out=out[m * 128:(m + 1) * 128, g * group:(g + 1) * group],
                in_=oev[:],
            )

    pending_c = None
    if True:
        if True:
            for g in range(ng):
                if g % gpb == 0:
                    # scan state resets at each batch boundary
                    nc.vector.memset(h_sb[:], 0.0)

                gs = slice(g * group, (g + 1) * group)
                hid_t = []
                for hh in range(2):
                    if x_fp8:
                        ht = p_hid.tile([128, kh, group], dt.float8e4, tag="hidlo")
                        nc.sync.dma_start(
                            out=ht[:], in_=hidlo_r[:, hh * kh:(hh + 1) * kh, gs],
                        )
                    else:
                        ht = p_hid.tile([128, kh, group], dt.bfloat16, tag="hid")
                        nc.sync.dma_start(
                            out=ht[:], in_=hid_r[:, hh * kh:(hh + 1) * kh, gs],
                        )
                    hid_t.append(ht)   # x_fp8: hid_t holds the LO activations
                hid8_t = []
                if z_fp8:
                    for hh in range(2):
                        ht8 = p_hid8.tile([128, kh, group], dt.float8e4, tag="hid8")
                        nc.sync.dma_start(
                            out=ht8[:], in_=hid8_r[:, hh * kh:(hh + 1) * kh, gs],
                        )
                        hid8_t.append(ht8)

                y_sb = p_y.tile([128, kc, group], dt.bfloat16)

                for s in range(kc):
                    px = p_psA.tile([128, group], dt.float32, tag="ps")
                    pz = p_psA.tile([128, group], dt.float32, tag="ps")

                    def emit_x():
                        if x_fp8:
                            # x = hi@Whi + hi@Wlo + lo@Whi (lo@Wlo dropped),
                            # all fp8 DoubleRow; hid_t holds the lo residuals.
                            terms = [(wxhi_sb, hid8_t), (wxlo_sb, hid8_t),
                                     (wxhi_sb, hid_t)]
                            n = len(terms)
                            for k2 in range(0, ka, 2):
                                hh, kk = divmod(k2, kh)
                                for t, (wsb, hsb) in enumerate(terms):
                                    nc.tensor.matmul(
                                        px,
                                        wsb[:, k2:k2 + 2, s * 128:(s + 1) * 128],
                                        hsb[hh][:, kk:kk + 2, :],
                                        start=(k2 == 0 and t == 0),
                                        stop=(k2 == ka - 2 and t == n - 1),
                                        perf_mode=mybir.MatmulPerfMode.DoubleRow,
                                    )
                            return
                        wsrc = wx_sb if z_fp8 else win_sb
                        for k in range(ka):
                            hh, kk = divmod(k, kh)
                            nc.tensor.matmul(
                                px,
                                wsrc[:, k, s * 128:(s + 1) * 128],
                                hid_t[hh][:, kk, :],
                                start=(k == 0),
                                stop=(k == ka - 1),
                            )

                    def emit_z():
                        if z_fp8:
                            # fp8 DoubleRow: one matmul covers 2 k-tiles
                            for k2 in range(0, ka, 2):
                                hh, kk = divmod(k2, kh)
                                nc.tensor.matmul(
                                    pz,
                                    wz_sb[:, k2:k2 + 2, s * 128:(s + 1) * 128],
                                    hid8_t[hh][:, kk:kk + 2, :],
                                    start=(k2 == 0),
                                    stop=(k2 == ka - 2),
                                    perf_mode=mybir.MatmulPerfMode.DoubleRow,
                                )
                        else:
                            for k in range(ka):
                                hh, kk = divmod(k, kh)
                                nc.tensor.matmul(
                                    pz,
                                    win_sb[:, k, isl + s * 128:isl + (s + 1) * 128],
                                    hid_t[hh][:, kk, :],
                                    start=(k == 0),
                                    stop=(k == ka - 1),
                                )

                    def emit_xz_interleaved():
                        # Pair two 213ns bf16 x-matmuls with one fp8 DoubleRow
                        # z-matmul (107ns compute, ~213ns LDWEIGHTS): the long
                        # z weight-load hides under the x matmuls and the PE
                        # never waits on the weight path.
                        for j in range(ka // 2):
                            for k in (2 * j, 2 * j + 1):
                                hh, kk = divmod(k, kh)
                                nc.tensor.matmul(
                                    px,
                                    wx_sb[:, k, s * 128:(s + 1) * 128],
                                    hid_t[hh][:, kk, :],
                                    start=(k == 0),
                                    stop=(k == ka - 1),
                                )
                            k2 = 2 * j
                            hh, kk = divmod(k2, kh)
                            nc.tensor.matmul(
                                pz,
                                wz_sb[:, k2:k2 + 2, s * 128:(s + 1) * 128],
                                hid8_t[hh][:, kk:kk + 2, :],
                                start=(k2 == 0),
                                stop=(k2 == ka - 2),
                                perf_mode=mybir.MatmulPerfMode.DoubleRow,
                            )

                    if xz_interleave:
                        emit_xz_interleaved()
                    elif z_first:
                        emit_z()
                        emit_x()
                    else:
                        emit_x()
                        emit_z()
                    sig = p_sig.tile([128, group], dt.bfloat16, tag="sig")
                    nc.scalar.activation(
                        sig[:], pz[:], mybir.ActivationFunctionType.Sigmoid,
                        scale=(1.0 / ZSCALE) if z_fp8 else 1.0,
                    )
                    for c in range(cpg):
                        cs = slice(c * chunk, (c + 1) * chunk)
                        nc.vector.scalar_tensor_tensor(
                            out=h_sb[:, s, :],
                            in0=h_sb[:, s, :],
                            scalar=a_sb[:, s:s + 1],
                            in1=px[:, cs],
                            op0=mybir.AluOpType.mult,
                            op1=mybir.AluOpType.add,
                        )
                        nc.vector.tensor_mul(y_sb[:, s, cs], h_sb[:, s, :], sig[:, cs])

                if pipeline:
                    # depth-1 software pipeline: group g-1's out_proj runs
                    # after group g's in_proj on the PE, so the PE never
                    # waits on the DVE scan/gate chain.
                    if pending_c is not None:
                        emit_phase_c(*pending_c)
                    pending_c = (g, y_sb)
                else:
                    emit_phase_c(g, y_sb)
            if pipeline and pending_c is not None:
                emit_phase_c(*pending_c)
                pending_c = None


def null_build(z_fp8=True, x_fp8=None):
    """Trivial kernel with the same I/O signature (per-call overhead cal)."""
    import concourse.bass as bass
    import concourse.tile as tile
    import concourse.mybir as mybir

    if x_fp8 is None:
        x_fp8 = USE_X_FP8
    _patch_tile_drain()
    dt = mybir.dt
    nc0 = bass.Bass("TRN2")
    if x_fp8:
        nc0.dram_tensor("wxhi8_t", (DM, ISL), dt.float8e4, kind="ExternalInput")
        nc0.dram_tensor("wxlo8_t", (DM, ISL), dt.float8e4, kind="ExternalInput")
        nc0.dram_tensor("hidlo8_t", (DM, T), dt.float8e4, kind="ExternalInput")
        nc0.dram_tensor("wz8_t", (DM, ISL), dt.float8e4, kind="ExternalInput")
        nc0.dram_tensor("hid8_t", (DM, T), dt.float8e4, kind="ExternalInput")
    elif z_fp8:
        nc0.dram_tensor("hid_t", (DM, T), dt.bfloat16, kind="ExternalInput")
        nc0.dram_tensor("wx_t", (DM, ISL), dt.bfloat16, kind="ExternalInput")
        nc0.dram_tensor("wz8_t", (DM, ISL), dt.float8e4, kind="ExternalInput")
        nc0.dram_tensor("hid8_t", (DM, T), dt.float8e4, kind="ExternalInput")
    else:
        nc0.dram_tensor("hid_t", (DM, T), dt.bfloat16, kind="ExternalInput")
        nc0.dram_tensor("w_in_t", (DM, 2 * ISL), dt.bfloat16, kind="ExternalInput")
    nc0.dram_tensor("w_out_t", (ISL, DM), dt.bfloat16, kind="ExternalInput")
    nc0.dram_tensor("a_vec", (128, ISL // 128), dt.float32, kind="ExternalInput")
    out0 = nc0.dram_tensor("out_t", (DM, T), dt.bfloat16, kind="ExternalOutput")
    with tile.TileContext(nc0) as tc:
        with tc.tile_pool(name="p", bufs=1) as p:
            z = p.tile([128, 16], dt.bfloat16)
            nc0.vector.memset(z[:], 0.0)
            nc0.sync.dma_start(out=out0[0:128, 0:16], in_=z[:])
    return nc0


# Results of the most recent device run (for test harness introspection).
last_result = None


def _prep_core_inputs(hidden_states, W_in, W_out, A_log, z_fp8=True,
                      x_fp8=None):
    """Host-side shard prep. Returns per-core in_maps."""
    if x_fp8 is None:
        x_fp8 = USE_X_FP8
    F8 = ml_dtypes.float8_e4m3
    hidT = np.ascontiguousarray(hidden_states.reshape(T, DM).T)
    hid_t = None if x_fp8 else hidT.astype(BF16)
    hid8_t = hidT.astype(F8) if z_fp8 else None
    if x_fp8:
        hidlo8_t = (
            hidT.astype(np.float32) - hid8_t.astype(np.float32)
        ).astype(F8)
    A_full = np.exp(-np.abs(A_log)).astype(np.float32)  # (H,)

    in_maps = []
    for c in range(N_CORES):
        isl_sl = slice(c * ISL, (c + 1) * ISL)
        # fold the 0.1 scan input scale into the x-rows of W_in
        w_x = W_in[:INNER][isl_sl] * np.float32(0.1)
        w_z = W_in[INNER:][isl_sl]
        # x_fp8: scan is linear in x, so scale W_x up into fp8 range and
        # fold the inverse scale into W_out (y carries the scale).
        wout_scale = np.float32(1.0 / XSCALE) if x_fp8 else np.float32(1.0)
        w_out_t = np.ascontiguousarray(
            (W_out[:, isl_sl] * wout_scale).T
        ).astype(BF16)  # (ISL, DM)
        a_col = np.repeat(A_full[c * HPC:(c + 1) * HPC], DH)  # (ISL,)
        a_vec = np.ascontiguousarray(
            a_col.reshape(ISL // 128, 128).T
        ).astype(np.float32)  # (128, ISL/128)
        if z_fp8 and x_fp8:
            ws = (w_x * np.float32(XSCALE)).astype(np.float32)
            whi = ws.astype(F8)
            wlo = (ws - whi.astype(np.float32)).astype(F8)
            m = {
                "hid8_t": hid8_t,
                "hidlo8_t": hidlo8_t,
                "wxhi8_t": np.ascontiguousarray(whi.T),
                "wxlo8_t": np.ascontiguousarray(wlo.T),
                "wz8_t": np.ascontiguousarray(
                    (w_z * np.float32(ZSCALE)).T
                ).astype(F8),
                "w_out_t": w_out_t,
                "a_vec": a_vec,
            }
        elif z_fp8:
            m = {
                "hid_t": hid_t,
                "hid8_t": hid8_t,
                "wx_t": np.ascontiguousarray(w_x.T).astype(BF16),
                "wz8_t": np.ascontiguousarray(
                    (w_z * np.float32(ZSCALE)).T
                ).astype(F8),
                "w_out_t": w_out_t,
                "a_vec": a_vec,
            }
        else:
            m = {
                "hid_t": hid_t,
                "w_in_t": np.ascontiguousarray(
                    np.concatenate([w_x, w_z], axis=0).T
                ).astype(BF16),
                "w_out_t": w_out_t,
                "a_vec": a_vec,
            }
        in_maps.append(m)
    return in_maps


def kernel(hidden_states, W_in, W_out, A_log):
    from concourse.bass_utils import run_bass_kernel_spmd

    global last_result

    if "nc" not in _nc_cache:
        _nc_cache["nc"] = _build_bass()
    nc = _nc_cache["nc"]

    hidden_states = np.asarray(hidden_states, dtype=np.float32)
    W_in = np.asarray(W_in, dtype=np.float32)
    W_out = np.asarray(W_out, dtype=np.float32)
    A_log = np.asarray(A_log, dtype=np.float32)

    in_maps = _prep_core_inputs(hidden_states, W_in, W_out, A_log)

    last_result = run_bass_kernel_spmd(nc, in_maps, core_ids=list(range(N_CORES)))

    acc = np.zeros((DM, T), dtype=np.float32)
    for r in last_result.results:
        acc += np.asarray(r["out_t"], dtype=np.float32)
    return np.ascontiguousarray(acc.T).reshape(B, S, DM)


if __name__ == "__main__":
    rng = np.random.default_rng(0)
    ins = {
        "hidden_states": rng.standard_normal((B, S, DM), dtype=np.float32),
        "W_in": (rng.standard_normal((2 * INNER, DM), dtype=np.float32) * 0.02),
        "W_out": (rng.standard_normal((DM, INNER), dtype=np.float32) * 0.02),
        "A_log": rng.standard_normal((H,), dtype=np.float32),
    }
    out = kernel(**ins)
    print(out.shape, out.dtype)



# revision 60
# speedup vs baseline: 1.1836x; 1.0475x over previous
"""Trainium2 Bass kernel for the FallbackSSMKernel problem.

Computation (reference):
    xz = hidden @ W_in.T                     # (B,S,2*INNER)
    x, z = split(xz);  x -> (B,S,H,DH)
    h_n = A*h_{n-1} + 0.1*x_n  over chunks of 256 positions (per head)
    y = scan_out * sigmoid(z)
    out = y @ W_out.T                        # (B,S,DM)

Sharding: 8-way tensor-parallel by heads (14 heads / 896 inner dims per
core).  Each core computes the full token range for its head slice and a
partial out-projection (contraction over its 896 inner dims); bf16
partials are summed in fp32 on the host.

Device layout is fully transposed: activations live as (feature, token)
with features on SBUF partitions, so no transposes are ever needed on
device.  Per 512-token group the kernel runs:
  A) in_proj: PSUM tiles (x_s/z_s pairs) accumulated over 28 K-tiles of
     128.  The x branch (feeds the scan, error-sensitive) uses bf16;
     the z branch (only feeds a sigmoid gate) uses fp8e4m3 matmuls in
     DoubleRow perf mode (2 K-tiles per matmul, 2x PE throughput).  W_z
     is pre-scaled by 64 on the host to clear the fp8 subnormal range
     and the 1/64 is folded into the sigmoid's input scale.
  B) scan+gate directly from PSUM: h = h*A + x (DVE scalar_tensor_tensor,
     fp32 state), sig = sigmoid(z/64) (ACT), y = h*sig -> bf16 SBUF
  C) out_proj (transposed): outT[dm_tile, tokens] accumulated over the 7
     inner K-tiles, evicted via DVE to bf16 and DMA'd to DRAM.

All load DMAs ride the SP HWDGE queue and stores ride it too (stores on
the ACT queue measured ~200us slower).  The 0.1 scan input scale is
folded into the x-rows of W_in on the host.  Measured end-to-end
relative error 0.0169 (gate 2e-2), dominated by the fp8 z branch;
validated bit-exactly against a CPU emulation of the same datapath.
"""

import numpy as np
import ml_dtypes

B, S, DM = 2, 4096, 3584
H, DH = 112, 64
CHUNK = 256
INNER = H * DH
N_CORES = 8
HPC = H // N_CORES          # heads per core = 14
ISL = HPC * DH              # inner slice per core = 896
T = B * S                   # total tokens = 8192
G = 512                     # tokens per group

BF16 = ml_dtypes.bfloat16

_nc_cache = {}


def _patch_tile_drain():
    """Split the Tile end-of-kernel drain's semaphore waits across NOPs.

    The walrus build here rejects an InstDrain carrying more than a
    couple of sync waits ("Too many sync wait commands" in
    CoreV3GenImpl::setupSyncWait).  TileContext._drain_and_barrier
    attaches one wait per outstanding logical processor to the single
    drain, which trips that limit for any kernel that used a few DMA
    queues.  Emit one single-wait NOP per processor first so the drain
    itself needs no waits.
    """
    import concourse.tile as tile
    from concourse.vector_clock import ScopedClock, VectorClock

    if getattr(tile.TileContext, "_drain_split_patched", False):
        return

    def _drain_and_barrier(self, tick_clock, wait_clock):
        full = tick_clock.global_clock
        n = len(full)
        for proc in range(n):
            t = full[proc]
            if t > 0:
                vec = [0] * n
                vec[proc] = t
                nop = self.nc.sync.nop(nofuse=True, hint="drain_split")
                wait_clock.add_sem_waits(nop.ins, ScopedClock({None: VectorClock(vec)}))
        # No waits on the drain itself: SP executes the single-wait NOPs
        # above in order first, so every processor's final tick has been
        # observed before the drain runs.
        self.nc.sync.drain()
        self.nc.all_engine_barrier()
        popped = self.nc._tile_sem_poison_stack.pop()
        assert popped is self._sem_poison
        self.nc.clear_and_free_semaphores(list(self.sems.allocated().values()))
        self.nc.all_engine_barrier()

    tile.TileContext._drain_and_barrier = _drain_and_barrier
    tile.TileContext._drain_split_patched = True


def _split_excess_waits(nc, limit=1):
    """Hoist excess per-instruction semaphore waits onto inserted NOPs.

    The TRN2 64-byte instruction encoding carries at most `limit` sync
    waits; this walrus build hard-errors on more.  Tile can attach 3+
    waits to one instruction.  Hoisting the earliest waits onto
    preceding same-engine NOPs is semantics-preserving: semaphore
    values are monotonic, so waiting earlier on the same engine keeps
    the ordering guarantees.
    """
    import concourse.mybir as mybir

    counter = [0]
    for f in nc.m.functions:
        for blk in f.blocks:
            insts = blk.instructions
            new = []
            changed = False
            for inst in insts:
                si = inst.sync_info
                if si is not None and si.on_wait and len(si.on_wait) > limit:
                    waits = list(si.on_wait)
                    extra, keep = waits[:-limit], waits[-limit:]
                    for i in range(0, len(extra), limit):
                        chunk_w = extra[i:i + limit]
                        nop = mybir.InstNoOp(
                            name=f"WSPLIT-{counter[0]}", ins=[], outs=[]
                        )
                        counter[0] += 1
                        nop.engine = inst.engine
                        nop.sync_info = mybir.SyncInfo(
                            on_wait=chunk_w, on_update=[]
                        )
                        new.append(nop)
                    si.on_wait = keep
                    changed = True
                new.append(inst)
            if changed:
                blk.instructions = new
    return counter[0]


ZSCALE = 64.0     # z-branch fp8 weight pre-scale (folded out in the sigmoid)
XSCALE = 1024.0   # x-branch fp8 weight pre-scale (folded out through W_out)
# x-branch as fp8 hi/lo DoubleRow (3 matmuls per k-pair).  Measured ~480us
# SLOWER than bf16 on HW: DoubleRow disables fast-weight-load, and 42
# LDWEIGHTS of ~213ns per slab exceed the matmul savings.  Keep off.
USE_X_FP8 = False


def _build_bass(dm=DM, isl=ISL, tokens=T, n_batch=B, group=G, chunk=CHUNK,
                repeat=1, psa_bufs=4, psc_bufs=4, z_first=False,
                store_eng="sp", z_fp8=True, xz_interleave=False, oev_bufs=4,
                x_fp8=None, pipeline=False, evict_eng="dve"):
    """Build the per-core Bass module.

    Inputs (per core):
      hid_t  (dm, tokens)  bf16 : hidden_states, transposed
      w_in_t (dm, 2*isl)   bf16 : in_proj weight shard, transposed;
                                  cols [0,isl) are x-rows (pre-scaled by
                                  0.1), cols [isl,2*isl) are z-rows
      w_out_t(isl, dm)     bf16 : out_proj weight shard, transposed
      a_vec  (128, isl/128) f32 : per-inner-dim decay A = exp(-|A_log|)
    Output:
      out_t  (dm, tokens)  f32 : partial out-projection, transposed
    """
    import concourse.bass as bass
    import concourse.tile as tile
    import concourse.mybir as mybir

    _patch_tile_drain()

    ka = dm // 128            # in_proj contraction tiles
    kc = isl // 128           # out_proj contraction tiles / x slabs
    ng = tokens // group      # token groups
    gpb = ng // n_batch       # groups per batch
    kh = ka // 2              # k-tiles per hidden half-slab
    cpg = group // chunk      # chunks per group

    nc = bass.Bass("TRN2")
    dt = mybir.dt

    if x_fp8 is None:
        x_fp8 = USE_X_FP8
    assert not (x_fp8 and not z_fp8)
    hid_r = hidlo_r = wxhi_r = wxlo_r = None
    if not x_fp8:
        hid = nc.dram_tensor("hid_t", (dm, tokens), dt.bfloat16, kind="ExternalInput")
        hid_r = hid[:].rearrange("(k p) t -> p k t", p=128)
    if z_fp8:
        if x_fp8:
            # x-branch hi/lo fp8: hid8 doubles as the hi activation part
            wxhi = nc.dram_tensor("wxhi8_t", (dm, isl), dt.float8e4, kind="ExternalInput")
            wxlo = nc.dram_tensor("wxlo8_t", (dm, isl), dt.float8e4, kind="ExternalInput")
            hidlo = nc.dram_tensor("hidlo8_t", (dm, tokens), dt.float8e4, kind="ExternalInput")
            wxhi_r = wxhi[:].rearrange("(k p) m -> p k m", p=128)
            wxlo_r = wxlo[:].rearrange("(k p) m -> p k m", p=128)
            hidlo_r = hidlo[:].rearrange("(k p) t -> p k t", p=128)
        else:
            wx = nc.dram_tensor("wx_t", (dm, isl), dt.bfloat16, kind="ExternalInput")
            wx_r = wx[:].rearrange("(k p) m -> p k m", p=128)
        wz8 = nc.dram_tensor("wz8_t", (dm, isl), dt.float8e4, kind="ExternalInput")
        hid8 = nc.dram_tensor("hid8_t", (dm, tokens), dt.float8e4, kind="ExternalInput")
        wz8_r = wz8[:].rearrange("(k p) m -> p k m", p=128)
        hid8_r = hid8[:].rearrange("(k p) t -> p k t", p=128)
    else:
        w_in = nc.dram_tensor("w_in_t", (dm, 2 * isl), dt.bfloat16, kind="ExternalInput")
        win_r = w_in[:].rearrange("(k p) m -> p k m", p=128)
    w_out = nc.dram_tensor("w_out_t", (isl, dm), dt.bfloat16, kind="ExternalInput")
    a_vec = nc.dram_tensor("a_vec", (128, kc), dt.float32, kind="ExternalInput")
    # bf16 partials: halves store-DMA traffic; host sums 8 partials in fp32
    out = nc.dram_tensor("out_t", (dm, tokens), dt.bfloat16, kind="ExternalOutput")
    wout_r = w_out[:].rearrange("(k p) n -> p k n", p=128)

    with tile.TileContext(nc) as tc:
        with (
            tc.tile_pool(name="w_in", bufs=1) as p_win,
            tc.tile_pool(name="w_z8", bufs=1) as p_wz,
            tc.tile_pool(name="w_out", bufs=1) as p_wout,
            tc.tile_pool(name="consts", bufs=1) as p_const,
            tc.tile_pool(name="hid", bufs=2) as p_hid,
            tc.tile_pool(name="hid8", bufs=2) as p_hid8,
            tc.tile_pool(name="ysb", bufs=2) as p_y,
            tc.tile_pool(name="hstate", bufs=1) as p_h,
            tc.tile_pool(name="sig", bufs=4) as p_sig,
            tc.tile_pool(name="oev", bufs=oev_bufs) as p_oev,
            tc.tile_pool(name="psA", bufs=psa_bufs, space="PSUM") as p_psA,
            tc.tile_pool(name="psC", bufs=psc_bufs, space="PSUM") as p_psC,
        ):
            # Loads ride the SP HWDGE queue; wout + all phase-C stores ride
            # the ACT HWDGE queue so stores never head-of-line-block loads.
            st = nc.scalar if store_eng == "act" else nc.sync
            a_sb = p_const.tile([128, kc], dt.float32)
            nc.sync.dma_start(out=a_sb[:], in_=a_vec[:])
            if z_fp8 and x_fp8:
                wxhi_sb = p_win.tile([128, ka, isl], dt.float8e4)
                wxlo_sb = p_win.tile([128, ka, isl], dt.float8e4)
                for k in range(ka):
                    nc.sync.dma_start(out=wxhi_sb[:, k, :], in_=wxhi_r[:, k, :])
                    nc.sync.dma_start(out=wxlo_sb[:, k, :], in_=wxlo_r[:, k, :])
                wz_sb = p_wz.tile([128, ka, isl], dt.float8e4)
                for k in range(ka):
                    nc.sync.dma_start(out=wz_sb[:, k, :], in_=wz8_r[:, k, :])
                win_sb = ((wxhi_sb, wxlo_sb), wz_sb)
            elif z_fp8:
                wx_sb = p_win.tile([128, ka, isl], dt.bfloat16)
                for k in range(ka):
                    nc.sync.dma_start(out=wx_sb[:, k, :], in_=wx_r[:, k, :])
                wz_sb = p_wz.tile([128, ka, isl], dt.float8e4)
                for k in range(ka):
                    nc.sync.dma_start(out=wz_sb[:, k, :], in_=wz8_r[:, k, :])
                win_sb = (wx_sb, wz_sb)
            else:
                win_sb = p_win.tile([128, ka, 2 * isl], dt.bfloat16)
                for k in range(ka):
                    nc.sync.dma_start(out=win_sb[:, k, :], in_=win_r[:, k, :])
            wout_sb = p_wout.tile([128, kc, dm], dt.bfloat16)
            for k in range(kc):
                st.dma_start(out=wout_sb[:, k, :], in_=wout_r[:, k, :])
            h_sb = p_h.tile([128, kc, chunk], dt.float32)

            import contextlib
            rep_ctx = (
                tc.For_i(0, repeat, 1) if repeat > 1 else contextlib.nullcontext()
            )
            with rep_ctx:
                _emit_groups(
                    nc, tc, mybir, ng, gpb, ka, kh, kc, cpg, dm, isl, group, chunk,
                    hid_r, out, p_hid, p_y, p_sig, p_oev, p_psA, p_psC,
                    win_sb, wout_sb, a_sb, h_sb, z_first=z_first, store=st,
                    hid8_r=hid8_r if z_fp8 else None,
                    p_hid8=p_hid8 if z_fp8 else None,
                    xz_interleave=xz_interleave and z_fp8,
                    hidlo_r=hidlo_r, pipeline=pipeline, evict_eng=evict_eng,
                )

    n_split = _split_excess_waits(nc)
    if n_split:
        print(f"_split_excess_waits: inserted {n_split} NOPs", flush=True)
    return nc


def _emit_groups(nc, tc, mybir, ng, gpb, ka, kh, kc, cpg, dm, isl, group, chunk,
                 hid_r, out, p_hid, p_y, p_sig, p_oev, p_psA, p_psC,
                 win_sb, wout_sb, a_sb, h_sb, pipeline=False, z_first=False,
                 store=None, hid8_r=None, p_hid8=None, xz_interleave=False,
                 hidlo_r=None, evict_eng="dve"):
    z_fp8 = hid8_r is not None
    x_fp8 = hidlo_r is not None
    if z_fp8:
        wx_sb, wz_sb = win_sb
        if x_fp8:
            wxhi_sb, wxlo_sb = wx_sb
    # pipeline=True: out_proj of group g-1 is emitted after in_proj of
    # group g, so the PE never stalls on the DVE scan/gate chain at the
    # A->C boundary.  No hang under the current schedule, but long-run
    # timing is bimodal on HW (1.88ms..2.38ms/iter across sessions,
    # likely a sustained-power/HAM interaction) while pipeline=False is
    # stable at ~2.02-2.06ms.  Keep off: stable beats occasionally-fast.
    dt = mybir.dt

    def emit_phase_c(g, y_sb):
        for m in range(dm // 128):
            po = p_psC.tile([128, group], dt.float32, tag="po")
            for k in range(kc):
                nc.tensor.matmul(
                    po,
                    wout_sb[:, k, m * 128:(m + 1) * 128],
                    y_sb[:, k, :],
                    start=(k == 0),
                    stop=(k == kc - 1),
                )
            oev = p_oev.tile([128, group], dt.bfloat16, tag="oev")
            if evict_eng == "act":
                nc.scalar.copy(oev[:], po[:])
            else:
                nc.vector.tensor_copy(oev[:], po[:])
            (store or nc.sync).dma_start(
                out=out[m * 128:(m + 1) * 128, g * group:(g + 1) * group],
                in_=oev[:],
            )

    pending_c = None
    if True:
        if True:
            for g in range(ng):
                if g % gpb == 0:
                    # scan state resets at each batch boundary
                    nc.vector.memset(h_sb[:], 0.0)

                gs = slice(g * group, (g + 1) * group)
                hid_t = []
                for hh in range(2):
                    if x_fp8:
                        ht = p_hid.tile([128, kh, group], dt.float8e4, tag="hidlo")
                        nc.sync.dma_start(
                            out=ht[:], in_=hidlo_r[:, hh * kh:(hh + 1) * kh, gs],
                        )
                    else:
                        ht = p_hid.tile([128, kh, group], dt.bfloat16, tag="hid")
                        nc.sync.dma_start(
                            out=ht[:], in_=hid_r[:, hh * kh:(hh + 1) * kh, gs],
                        )
                    hid_t.append(ht)   # x_fp8: hid_t holds the LO activations
                hid8_t = []
                if z_fp8:
                    for hh in range(2):
                        ht8 = p_hid8.tile([128, kh, group], dt.float8e4, tag="hid8")
                        nc.sync.dma_start(
                            out=ht8[:], in_=hid8_r[:, hh * kh:(hh + 1) * kh, gs],
                        )
                        hid8_t.append(ht8)

                y_sb = p_y.tile([128, kc, group], dt.bfloat16)

                for s in range(kc):
                    px = p_psA.tile([128, group], dt.float32, tag="ps")
                    pz = p_psA.tile([128, group], dt.float32, tag="ps")

                    def emit_x():
                        if x_fp8:
                            # x = hi@Whi + hi@Wlo + lo@Whi (lo@Wlo dropped),
                            # all fp8 DoubleRow; hid_t holds the lo residuals.
                            terms = [(wxhi_sb, hid8_t), (wxlo_sb, hid8_t),
                                     (wxhi_sb, hid_t)]
                            n = len(terms)
                            for k2 in range(0, ka, 2):
                                hh, kk = divmod(k2, kh)
                                for t, (wsb, hsb) in enumerate(terms):
                                    nc.tensor.matmul(
                                        px,
                                        wsb[:, k2:k2 + 2, s * 128:(s + 1) * 128],
                                        hsb[hh][:, kk:kk + 2, :],
                                        start=(k2 == 0 and t == 0),
                                        stop=(k2 == ka - 2 and t == n - 1),
                                        perf_mode=mybir.MatmulPerfMode.DoubleRow,
                                    )
                            return
                        wsrc = wx_sb if z_fp8 else win_sb
                        for k in range(ka):
                            hh, kk = divmod(k, kh)
                            nc.tensor.matmul(
                                px,
                                wsrc[:, k, s * 128:(s + 1) * 128],
                                hid_t[hh][:, kk, :],
                                start=(k == 0),
                                stop=(k == ka - 1),
                            )

                    def emit_z():
                        if z_fp8:
                            # fp8 DoubleRow: one matmul covers 2 k-tiles
                            for k2 in range(0, ka, 2):
                                hh, kk = divmod(k2, kh)
                                nc.tensor.matmul(
                                    pz,
                                    wz_sb[:, k2:k2 + 2, s * 128:(s + 1) * 128],
                                    hid8_t[hh][:, kk:kk + 2, :],
                                    start=(k2 == 0),
                                    stop=(k2 == ka - 2),
                                    perf_mode=mybir.MatmulPerfMode.DoubleRow,
                                )
                        else:
                            for k in range(ka):
                                hh, kk = divmod(k, kh)
                                nc.tensor.matmul(
                                    pz,
                                    win_sb[:, k, isl + s * 128:isl + (s + 1) * 128],
                                    hid_t[hh][:, kk, :],
                                    start=(k == 0),
                                    stop=(k == ka - 1),
                                )

                    def emit_xz_interleaved():
                        # Pair two 213ns bf16 x-matmuls with one fp8 DoubleRow
                        # z-matmul (107ns compute, ~213ns LDWEIGHTS): the long
                        # z weight-load hides under the x matmuls and the PE
                        # never waits on the weight path.
                        for j in range(ka // 2):
                            for k in (2 * j, 2 * j + 1):
                                hh, kk = divmod(k, kh)
                                nc.tensor.matmul(
                                    px,
                                    wx_sb[:, k, s * 128:(s + 1) * 128],
                                    hid_t[hh][:, kk, :],
                                    start=(k == 0),
                                    stop=(k == ka - 1),
                                )
                            k2 = 2 * j
                            hh, kk = divmod(k2, kh)
                            nc.tensor.matmul(
                                pz,
                                wz_sb[:, k2:k2 + 2, s * 128:(s + 1) * 128],
                                hid8_t[hh][:, kk:kk + 2, :],
                                start=(k2 == 0),
                                stop=(k2 == ka - 2),
                                perf_mode=mybir.MatmulPerfMode.DoubleRow,
                            )

                    if xz_interleave:
                        emit_xz_interleaved()
                    elif z_first:
                        emit_z()
                        emit_x()
                    else:
                        emit_x()
                        emit_z()
                    sig = p_sig.tile([128, group], dt.bfloat16, tag="sig")
                    nc.scalar.activation(
                        sig[:], pz[:], mybir.ActivationFunctionType.Sigmoid,
                        scale=(1.0 / ZSCALE) if z_fp8 else 1.0,
                    )
                    for c in range(cpg):
                        cs = slice(c * chunk, (c + 1) * chunk)
                        nc.vector.scalar_tensor_tensor(
                            out=h_sb[:, s, :],
                            in0=h_sb[:, s, :],
                            scalar=a_sb[:, s:s + 1],
                            in1=px[:, cs],
                            op0=mybir.AluOpType.mult,
                            op1=mybir.AluOpType.add,
                        )
                        nc.vector.tensor_mul(y_sb[:, s, cs], h_sb[:, s, :], sig[:, cs])

                if pipeline:
                    # depth-1 software pipeline: group g-1's out_proj runs
                    # after group g's in_proj on the PE, so the PE never
                    # waits on the DVE scan/gate chain.
                    if pending_c is not None:
                        emit_phase_c(*pending_c)
                    pending_c = (g, y_sb)
                else:
                    emit_phase_c(g, y_sb)
            if pipeline and pending_c is not None:
                emit_phase_c(*pending_c)
                pending_c = None


def null_build(z_fp8=True, x_fp8=None):
    """Trivial kernel with the same I/O signature (per-call overhead cal)."""
    import concourse.bass as bass
    import concourse.tile as tile
    import concourse.mybir as mybir

    if x_fp8 is None:
        x_fp8 = USE_X_FP8
    _patch_tile_drain()
    dt = mybir.dt
    nc0 = bass.Bass("TRN2")
    if x_fp8:
        nc0.dram_tensor("wxhi8_t", (DM, ISL), dt.float8e4, kind="ExternalInput")
        nc0.dram_tensor("wxlo8_t", (DM, ISL), dt.float8e4, kind="ExternalInput")
        nc0.dram_tensor("hidlo8_t", (DM, T), dt.float8e4, kind="ExternalInput")
        nc0.dram_tensor("wz8_t", (DM, ISL), dt.float8e4, kind="ExternalInput")
        nc0.dram_tensor("hid8_t", (DM, T), dt.float8e4, kind="ExternalInput")
    elif z_fp8:
        nc0.dram_tensor("hid_t", (DM, T), dt.bfloat16, kind="ExternalInput")
        nc0.dram_tensor("wx_t", (DM, ISL), dt.bfloat16, kind="ExternalInput")
        nc0.dram_tensor("wz8_t", (DM, ISL), dt.float8e4, kind="ExternalInput")
        nc0.dram_tensor("hid8_t", (DM, T), dt.float8e4, kind="ExternalInput")
    else:
        nc0.dram_tensor("hid_t", (DM, T), dt.bfloat16, kind="ExternalInput")
        nc0.dram_tensor("w_in_t", (DM, 2 * ISL), dt.bfloat16, kind="ExternalInput")
    nc0.dram_tensor("w_out_t", (ISL, DM), dt.bfloat16, kind="ExternalInput")
    nc0.dram_tensor("a_vec", (128, ISL // 128), dt.float32, kind="ExternalInput")
    out0 = nc0.dram_tensor("out_t", (DM, T), dt.bfloat16, kind="ExternalOutput")
    with tile.TileContext(nc0) as tc:
        with tc.tile_pool(name="p", bufs=1) as p:
            z = p.tile([128, 16], dt.bfloat16)
            nc0.vector.memset(z[:], 0.0)
            nc0.sync.dma_start(out=out0[0:128, 0:16], in_=z[:])
    return nc0


# Results of the most recent device run (for test harness introspection).
last_result = None


def _prep_core_inputs(hidden_states, W_in, W_out, A_log, z_fp8=True,
                      x_fp8=None):
    """Host-side shard prep. Returns per-core in_maps."""
    if x_fp8 is None:
        x_fp8 = USE_X_FP8
    F8 = ml_dtypes.float8_e4m3
    hidT = np.ascontiguousarray(hidden_states.reshape(T, DM).T)
    hid_t = None if x_fp8 else hidT.astype(BF16)
    hid8_t = hidT.astype(F8) if z_fp8 else None
    if x_fp8:
        hidlo8_t = (
            hidT.astype(np.float32) - hid8_t.astype(np.float32)
        ).astype(F8)
    A_full = np.exp(-np.abs(A_log)).astype(np.float32)  # (H,)

    in_maps = []
    for c in range(N_CORES):
        isl_sl = slice(c * ISL, (c + 1) * ISL)
        # fold the 0.1 scan input scale into the x-rows of W_in
        w_x = W_in[:INNER][isl_sl] * np.float32(0.1)
        w_z = W_in[INNER:][isl_sl]
        # x_fp8: scan is linear in x, so scale W_x up into fp8 range and
        # fold the inverse scale into W_out (y carries the scale).
        wout_scale = np.float32(1.0 / XSCALE) if x_fp8 else np.float32(1.0)
        w_out_t = np.ascontiguousarray(
            (W_out[:, isl_sl] * wout_scale).T
        ).astype(BF16)  # (ISL, DM)
        a_col = np.repeat(A_full[c * HPC:(c + 1) * HPC], DH)  # (ISL,)
        a_vec = np.ascontiguousarray(
            a_col.reshape(ISL // 128, 128).T
        ).astype(np.float32)  # (128, ISL/128)
        if z_fp8 and x_fp8:
            ws = (w_x * np.float32(XSCALE)).astype(np.float32)
            whi = ws.astype(F8)
            wlo = (ws - whi.astype(np.float32)).astype(F8)
            m = {
                "hid8_t": hid8_t,
                "hidlo8_t": hidlo8_t,
                "wxhi8_t": np.ascontiguousarray(whi.T),
                "wxlo8_t": np.ascontiguousarray(wlo.T),
                "wz8_t": np.ascontiguousarray(
                    (w_z * np.float32(ZSCALE)).T
                ).astype(F8),
                "w_out_t": w_out_t,
                "a_vec": a_vec,
            }
        elif z_fp8:
            m = {
                "hid_t": hid_t,
                "hid8_t": hid8_t,
                "wx_t": np.ascontiguousarray(w_x.T).astype(BF16),
                "wz8_t": np.ascontiguousarray(
                    (w_z * np.float32(ZSCALE)).T
                ).astype(F8),
                "w_out_t": w_out_t,
                "a_vec": a_vec,
            }
        else:
            m = {
                "hid_t": hid_t,
                "w_in_t": np.ascontiguousarray(
                    np.concatenate([w_x, w_z], axis=0).T
                ).astype(BF16),
                "w_out_t": w_out_t,
                "a_vec": a_vec,
            }
        in_maps.append(m)
    return in_maps


def kernel(hidden_states, W_in, W_out, A_log):
    from concourse.bass_utils import run_bass_kernel_spmd

    global last_result

    if "nc" not in _nc_cache:
        _nc_cache["nc"] = _build_bass()
    nc = _nc_cache["nc"]

    hidden_states = np.asarray(hidden_states, dtype=np.float32)
    W_in = np.asarray(W_in, dtype=np.float32)
    W_out = np.asarray(W_out, dtype=np.float32)
    A_log = np.asarray(A_log, dtype=np.float32)

    in_maps = _prep_core_inputs(hidden_states, W_in, W_out, A_log)

    last_result = run_bass_kernel_spmd(nc, in_maps, core_ids=list(range(N_CORES)))

    acc = np.zeros((DM, T), dtype=np.float32)
    for r in last_result.results:
        acc += np.asarray(r["out_t"], dtype=np.float32)
    return np.ascontiguousarray(acc.T).reshape(B, S, DM)


if __name__ == "__main__":
    rng = np.random.default_rng(0)
    ins = {
        "hidden_states": rng.standard_normal((B, S, DM), dtype=np.float32),
        "W_in": (rng.standard_normal((2 * INNER, DM), dtype=np.float32) * 0.02),
        "W_out": (rng.standard_normal((DM, INNER), dtype=np.float32) * 0.02),
        "A_log": rng.standard_normal((H,), dtype=np.float32),
    }
    out = kernel(**ins)
    print(out.shape, out.dtype)

